# revision 17
# baseline (speedup 1.0000x reference)
"""Single-head AttentionBlock (B=4, N=2048, C=1024) on 8 TRN2 NeuronCores.

Key-split sharding: core c handles batch b=c//2 and KEY half kh=c%2.  Each
core computes K^T and V for its 1024 keys only, Q for ALL 2048 query rows,
then unnormalized E = exp(S_half) and O_half = E @ V_half.  The softmax
normalization merges exactly on the host: row sums add across the pair,
attention columns concatenate, partial outputs add then divide.  No
cross-core communication and no duplicated K/V projections.

The host feeds X^T with the core's own key half as columns 0:1024 (xta)
and the partner's as 1024:2048 (xtb); query rows are processed in the same
local order and un-permuted on the host.

All matmuls run in fp32r (full PE rate, ~13-bit operand rounding).
Layouts on chip (partition dim first):
  XTA/XTB = X_b^T halves [c=1024, n=1024]
  KT  = Wk@X^T[:,mine]    [d=1024, nloc=1024]  resident (32KB/part)
  V   = X[mine]@Wv^T      [nloc=1024, d=1024]  resident (32KB/part)
  QT  = Wq@X^T /sqrt d    [d=1024, q=2048]     resident (64KB/part)
  S   = QT.T @ KT         [q, nloc]  -> E = exp(S) from PSUM + row sums
  AT  = PE-transpose(E)   [nloc, q]            (lhsT for out)
  O   = AT.T @ V          [q, d]               unnormalized
bq (pre-scaled by 1/sqrt(d)) and bk are applied via ACT bias on the QT/KT
PSUM->SBUF copies; bv is added on the host.

Everything is SBUF-resident after phase 1 (no DRAM scratch at all).  The
weight pool recycles wk -> wv -> wq(A/B); loads ride Sync+Scalar HWDGE
queues, stores ride Scalar; E/O stores stream per chunk.  Phase 2 is
software-pipelined: iteration i emits S_i/exp_i, then AT_{i-1}/O_{i-1},
so the PE always has work while the (short) exp chain runs on ACT/DVE.
"""
import os

os.environ.pop("JAX_PLATFORMS", None)

from contextlib import ExitStack

import numpy as np

import concourse.bass as bass
import concourse.mybir as mybir
import concourse.tile as tile
from concourse import bacc
from concourse.bass_utils import run_bass_kernel_spmd
from concourse.masks import make_identity

B, N, C = 4, 2048, 1024
NH = N // 2          # key half per core / local key count
P = 128              # partitions
CC = C // P          # 8 contraction chunks
DC = C // P          # 8 d chunks
QC = N // P          # 16 query chunks per core (all rows)
NK8 = NH // P        # 8 local key chunks
F32 = mybir.dt.float32
F32R = mybir.dt.float32r
EXP = mybir.ActivationFunctionType.Exp
IDENT = mybir.ActivationFunctionType.Identity

_cached = {}
_last_in_maps = None


def _build():
    nc = bacc.Bacc("TRN2", target_bir_lowering=False, debug=False)

    xta_d = nc.dram_tensor("xta", [C, NH], F32R, kind="ExternalInput").ap()
    xtb_d = nc.dram_tensor("xtb", [C, NH], F32R, kind="ExternalInput").ap()
    wqt_d = nc.dram_tensor("wqt", [C, C], F32R, kind="ExternalInput").ap()
    wkt_d = nc.dram_tensor("wkt", [C, C], F32R, kind="ExternalInput").ap()
    wvt_d = nc.dram_tensor("wvt", [C, C], F32R, kind="ExternalInput").ap()
    bqs_d = nc.dram_tensor("bqs", [P, DC], F32, kind="ExternalInput").ap()
    bks_d = nc.dram_tensor("bks", [P, DC], F32, kind="ExternalInput").ap()
    e_d = nc.dram_tensor("e", [N, NH], F32R, kind="ExternalOutput").ap()
    s_d = nc.dram_tensor("s", [P, QC], F32, kind="ExternalOutput").ap()
    o_d = nc.dram_tensor("o", [N, C], F32, kind="ExternalOutput").ap()

    with tile.TileContext(nc) as tc:
        with (
            tc.tile_pool(name="consts", bufs=1) as consts,
            tc.tile_pool(name="kt", bufs=1) as kt_pool,
            tc.tile_pool(name="v", bufs=1) as v_pool,
            tc.tile_pool(name="qtc", bufs=4) as qtc_pool,
        ):
            xt_stack = ExitStack()
            xta_pool = xt_stack.enter_context(tc.tile_pool(name="xta", bufs=1))
            xtb_pool = xt_stack.enter_context(tc.tile_pool(name="xtb", bufs=1))
            w_pool = xt_stack.enter_context(tc.tile_pool(name="w", bufs=CC))

            ident = consts.tile([P, P], F32R, tag="ident", bufs=1)
            identf = consts.tile([P, P], F32, tag="identf", bufs=1)
            make_identity(nc, identf[:])
            nc.scalar.activation(ident[:], identf[:], IDENT, bias=0.0)
            bq_sb = consts.tile([P, DC], F32, tag="bq", bufs=1)
            nc.sync.dma_start(bq_sb[:], bqs_d[:])
            bk_sb = consts.tile([P, DC], F32, tag="bk", bufs=1)
            nc.sync.dma_start(bk_sb[:], bks_d[:])
            s_all = consts.tile([P, QC], F32, tag="s_all", bufs=1)

            kt_tiles = [kt_pool.tile([P, NH], F32R, name=f"kt{d}") for d in range(DC)]
            v_tiles = [v_pool.tile([P, C], F32R, name=f"v{n}") for n in range(NK8)]

            # loads: own key/query half + wk first (feeds K immediately)
            xta_tiles = []
            for cchunk in range(CC):
                t = xta_pool.tile([P, NH], F32R, name=f"xta{cchunk}")
                nc.sync.dma_start(t[:], xta_d[cchunk * P : (cchunk + 1) * P, :])
                xta_tiles.append(t)
            wk_tiles = []
            for cchunk in range(CC):
                t = w_pool.tile([P, C], F32R, name=f"wk{cchunk}", tag="w")
                nc.sync.dma_start(t[:], wkt_d[cchunk * P : (cchunk + 1) * P, :])
                wk_tiles.append(t)
            late_stack = ExitStack()
            dram = late_stack.enter_context(
                tc.tile_pool(name="dram", bufs=1, space="DRAM")
            )
            qt_scr = [
                dram.tile([P, DC * 512], F32R, name=f"qtscr{qb}") for qb in range(4)
            ]
            qt_scr3 = [t[:].rearrange("p (d n) -> p d n", d=DC) for t in qt_scr]
            bounceq = xt_stack.enter_context(tc.tile_pool(name="bounceq", bufs=4))

            ps1 = late_stack.enter_context(
                tc.tile_pool(name="ps1", bufs=8, space="PSUM")
            )
            if True:
                # ---- K: KT[d, nloc] = WkT.T @ XTA (+bk), resident
                for d in range(DC):
                    pts = [
                        ps1.tile([P, 512], F32, name="p_k", tag="ps1")
                        for _ in range(2)
                    ]
                    for cchunk in range(CC):
                        for nb in range(2):
                            nc.tensor.matmul(
                                pts[nb][:],
                                wk_tiles[cchunk][:, d * P : (d + 1) * P],
                                xta_tiles[cchunk][:, nb * 512 : (nb + 1) * 512],
                                start=(cchunk == 0),
                                stop=(cchunk == CC - 1),
                            )
                    for nb in range(2):
                        nc.scalar.activation(
                            kt_tiles[d][:, nb * 512 : (nb + 1) * 512],
                            pts[nb][:],
                            IDENT,
                            bias=bk_sb[:, d : d + 1],
                        )

                # ---- V: V[nloc, d] = XTA.T @ WvT, resident
                wv_tiles = []
                for cchunk in range(CC):
                    t = w_pool.tile([P, C], F32R, name=f"wv{cchunk}", tag="w")
                    nc.sync.dma_start(t[:], wvt_d[cchunk * P : (cchunk + 1) * P, :])
                    wv_tiles.append(t)
                for n in range(NK8):
                    pts = [
                        ps1.tile([P, 512], F32, name="p_v", tag="ps1")
                        for _ in range(2)
                    ]
                    for cchunk in range(CC):
                        for db in range(2):
                            nc.tensor.matmul(
                                pts[db][:],
                                xta_tiles[cchunk][:, n * P : (n + 1) * P],
                                wv_tiles[cchunk][:, db * 512 : (db + 1) * 512],
                                start=(cchunk == 0),
                                stop=(cchunk == CC - 1),
                            )
                    for db in range(2):
                        nc.vector.tensor_copy(
                            v_tiles[n][:, db * 512 : (db + 1) * 512], pts[db][:]
                        )

                # ---- Q: QT[d, qloc] = WqT.T @ [XTA | XTB] (+bq), resident
                wq_tiles = []
                for cchunk in range(CC):
                    t = w_pool.tile([P, C], F32R, name=f"wq{cchunk}", tag="w")
                    nc.sync.dma_start(t[:], wqt_d[cchunk * P : (cchunk + 1) * P, :])
                    wq_tiles.append(t)
                xtb_tiles = []
                for cchunk in range(CC):
                    t = xtb_pool.tile([P, NH], F32R, name=f"xtb{cchunk}")
                    nc.sync.dma_start(t[:], xtb_d[cchunk * P : (cchunk + 1) * P, :])
                    xtb_tiles.append(t)
                qtc_prefetch = []

                def load_qtc(qc):
                    qb, rel = divmod(qc, 4)
                    t = qtc_pool.tile([P, DC * P], F32R, name="qtc", tag="qtc")
                    nc.sync.dma_start(
                        t[:].rearrange("p (d n) -> p d n", d=DC),
                        qt_scr3[qb][:, :, rel * P : (rel + 1) * P],
                    )
                    return t

                for qb in range(4):
                    x_tiles = xta_tiles if qb < 2 else xtb_tiles
                    qrel = qb % 2
                    pts = [
                        ps1.tile([P, 512], F32, name="p_q", tag="ps1")
                        for _ in range(DC)
                    ]
                    for cchunk in range(CC):
                        for d in range(DC):
                            nc.tensor.matmul(
                                pts[d][:],
                                wq_tiles[cchunk][:, d * P : (d + 1) * P],
                                x_tiles[cchunk][:, qrel * 512 : (qrel + 1) * 512],
                                start=(cchunk == 0),
                                stop=(cchunk == CC - 1),
                            )
                    for d in range(DC):
                        qb_t = bounceq.tile([P, 512], F32R, name="qb_t", tag="bq_t")
                        nc.scalar.activation(
                            qb_t[:], pts[d][:], IDENT, bias=bq_sb[:, d : d + 1]
                        )
                        nc.gpsimd.dma_start(
                            qt_scr[qb][:, d * 512 : (d + 1) * 512], qb_t[:]
                        )
                    if qb == 0:
                        qtc_prefetch = [load_qtc(qc) for qc in range(3)]

            xt_stack.close()

            # ---------- Phase 2: attention, software-pipelined ----------
            with (
                tc.tile_pool(name="a", bufs=4) as a_pool,
                tc.tile_pool(name="atsb", bufs=8) as at_pool,
                tc.tile_pool(name="osb", bufs=3) as o_pool,
                tc.tile_pool(name="small", bufs=16) as small,
            ):
                ps_s = ps_at = ps_o = ps1
                qtc_queue = list(qtc_prefetch)
                prev = None  # (a_sb of chunk i-1, qc index)

                def emit_at(a_sb):
                    at_tiles = []
                    for g in range(2):
                        pt = ps_at.tile([P, 512], F32R, name="p_at", tag="ps1")
                        for j in range(4):
                            kk = g * 4 + j
                            nc.tensor.transpose(
                                pt[:, j * P : (j + 1) * P],
                                a_sb[:, kk * P : (kk + 1) * P],
                                ident[:],
                            )
                        at_sb = at_pool.tile([P, 512], F32R, name="at_sb", tag="at")
                        nc.vector.tensor_copy(at_sb[:], pt[:])
                        at_tiles.append(at_sb)
                    return at_tiles

                def emit_o(at_tiles, qc):
                    o_sb = o_pool.tile([P, C], F32, name="o_sb", tag="o")
                    pts = [
                        ps_o.tile([P, 512], F32, name="p_o", tag="ps1")
                        for _ in range(2)
                    ]
                    for kk in range(NK8):
                        for db in range(2):
                            nc.tensor.matmul(
                                pts[db][:],
                                at_tiles[kk // 4][:, (kk % 4) * P : (kk % 4 + 1) * P],
                                v_tiles[kk][:, db * 512 : (db + 1) * 512],
                                start=(kk == 0),
                                stop=(kk == NK8 - 1),
                            )
                    for db in range(2):
                        nc.scalar.copy(o_sb[:, db * 512 : (db + 1) * 512], pts[db][:])
                    nc.scalar.dma_start(o_d[qc * P : (qc + 1) * P, :], o_sb[:])

                for qc in range(QC):
                    qtc_tiles = qtc_queue.pop(0)
                    # transposes of the previous chunk first: the S matmuls
                    # below cover their PSUM->SBUF copies on the DVE
                    at_prev = emit_at(prev[0]) if prev is not None else None

                    # S_half[q, nloc], exp straight out of PSUM with row sums
                    a_sb = a_pool.tile([P, NH], F32R, name="a_sb", tag="a")
                    s2 = small.tile([P, 2], F32, name="s2", tag="s2")
                    s_pts = [
                        ps_s.tile([P, 512], F32, name="p_s", tag="ps1")
                        for _ in range(2)
                    ]
                    for d in range(DC):
                        for nb in range(2):
                            nc.tensor.matmul(
                                s_pts[nb][:],
                                qtc_tiles[:, d * P : (d + 1) * P],
                                kt_tiles[d][:, nb * 512 : (nb + 1) * 512],
                                start=(d == 0),
                                stop=(d == DC - 1),
                            )
                    if qc + 3 < QC:
                        qtc_queue.append(load_qtc(qc + 3))
                    for nb in range(2):
                        nc.scalar.activation(
                            a_sb[:, nb * 512 : (nb + 1) * 512],
                            s_pts[nb][:],
                            EXP,
                            bias=0.0,
                            accum_out=s2[:, nb : nb + 1],
                        )
                    if at_prev is not None:
                        emit_o(at_prev, prev[1])
                    nc.vector.reduce_sum(
                        s_all[:, qc : qc + 1], s2[:], axis=mybir.AxisListType.X
                    )
                    nc.gpsimd.dma_start(e_d[qc * P : (qc + 1) * P, :], a_sb[:])
                    prev = (a_sb, qc)

                emit_o(emit_at(prev[0]), prev[1])
                nc.sync.dma_start(s_d[:], s_all[:])
            late_stack.close()

    nc.compile()
    return nc


def kernel(hidden_states, Wq, bq, Wk, bk, Wv, bv):
    x = np.asarray(hidden_states, dtype=np.float32)
    Wq = np.asarray(Wq, dtype=np.float32)
    Wk = np.asarray(Wk, dtype=np.float32)
    Wv = np.asarray(Wv, dtype=np.float32)
    bq = np.asarray(bq, dtype=np.float32)
    bk = np.asarray(bk, dtype=np.float32)
    bv = np.asarray(bv, dtype=np.float32)

    if "nc" not in _cached:
        _cached["nc"] = _build()
    nc = _cached["nc"]

    scale = np.float32(1.0 / np.sqrt(C))
    wqt = np.ascontiguousarray(Wq.T) * scale
    wkt = np.ascontiguousarray(Wk.T)
    wvt = np.ascontiguousarray(Wv.T)
    bqs = np.ascontiguousarray((bq * scale).reshape(DC, P).T)
    bks = np.ascontiguousarray(bk.reshape(DC, P).T)

    in_maps = []
    for core in range(8):
        b, kh = divmod(core, 2)
        xt = np.ascontiguousarray(x[b].T)
        mine = xt[:, kh * NH : (kh + 1) * NH]
        other = xt[:, (1 - kh) * NH : (2 - kh) * NH]
        in_maps.append(
            {
                "xta": np.ascontiguousarray(mine),
                "xtb": np.ascontiguousarray(other),
                "wqt": wqt,
                "wkt": wkt,
                "wvt": wvt,
                "bqs": bqs,
                "bks": bks,
            }
        )

    global _last_in_maps
    _last_in_maps = in_maps
    res = run_bass_kernel_spmd(nc, in_maps, core_ids=list(range(8)))

    out = np.empty((B, N, C), dtype=np.float32)
    attention = np.empty((B, N, N), dtype=np.float32)
    for b in range(B):
        r0 = res.results[2 * b]      # kh = 0: local rows = global rows
        r1 = res.results[2 * b + 1]  # kh = 1: local rows = [half1 | half0]
        # s_all[r, qc] holds the row sum for local row qc*128 + r
        s0 = r0["s"].T.reshape(N)
        s1loc = r1["s"].T.reshape(N)
        s1 = np.concatenate([s1loc[NH:], s1loc[:NH]])
        stot = s0 + s1
        e1 = np.concatenate([r1["e"][NH:], r1["e"][:NH]], axis=0)
        attention[b, :, :NH] = r0["e"] / stot[:, None]
        attention[b, :, NH:] = e1 / stot[:, None]
        o1 = np.concatenate([r1["o"][NH:], r1["o"][:NH]], axis=0)
        out[b] = (r0["o"] + o1) / stot[:, None] + bv[None, :]
    return (out, attention)


# revision 18
# speedup vs baseline: 1.0360x; 1.0360x over previous
"""Single-head AttentionBlock (B=4, N=2048, C=1024) on 8 TRN2 NeuronCores.

Key-split sharding: core c handles batch b=c//2 and KEY half kh=c%2.  Each
core computes K^T and V for its 1024 keys only, Q for ALL 2048 query rows,
then unnormalized E = exp(S_half) and O_half = E @ V_half.  The softmax
normalization merges exactly on the host: row sums add across the pair,
attention columns concatenate, partial outputs add then divide.  No
cross-core communication and no duplicated K/V projections.

The host feeds X^T with the core's own key half as columns 0:1024 (xta)
and the partner's as 1024:2048 (xtb); query rows are processed in the same
local order and un-permuted on the host.

All matmuls run in fp32r (full PE rate, ~13-bit operand rounding).
Layouts on chip (partition dim first):
  XTA/XTB = X_b^T halves [c=1024, n=1024]
  KT  = Wk@X^T[:,mine]    [d=1024, nloc=1024]  resident (32KB/part)
  V   = X[mine]@Wv^T      [nloc=1024, d=1024]  resident (32KB/part)
  QT  = Wq@X^T /sqrt d    [d=1024, q=2048]     resident (64KB/part)
  S   = QT.T @ KT         [q, nloc]  -> E = exp(S) from PSUM + row sums
  AT  = PE-transpose(E)   [nloc, q]            (lhsT for out)
  O   = AT.T @ V          [q, d]               unnormalized
bq (pre-scaled by 1/sqrt(d)) and bk are applied via ACT bias on the QT/KT
PSUM->SBUF copies; bv is added on the host.

Everything is SBUF-resident after phase 1 (no DRAM scratch at all).  The
weight pool recycles wk -> wv -> wq(A/B); loads ride Sync+Scalar HWDGE
queues, stores ride Scalar; E/O stores stream per chunk.  Phase 2 is
software-pipelined: iteration i emits S_i/exp_i, then AT_{i-1}/O_{i-1},
so the PE always has work while the (short) exp chain runs on ACT/DVE.
"""
import os

os.environ.pop("JAX_PLATFORMS", None)

from contextlib import ExitStack

import numpy as np

import concourse.bass as bass
import concourse.mybir as mybir
import concourse.tile as tile
from concourse import bacc
from concourse.bass_utils import run_bass_kernel_spmd
from concourse.masks import make_identity

B, N, C = 4, 2048, 1024
NH = N // 2          # key half per core / local key count
P = 128              # partitions
CC = C // P          # 8 contraction chunks
DC = C // P          # 8 d chunks
QC = N // P          # 16 query chunks per core (all rows)
NK8 = NH // P        # 8 local key chunks
F32 = mybir.dt.float32
F32R = mybir.dt.float32r
EXP = mybir.ActivationFunctionType.Exp
IDENT = mybir.ActivationFunctionType.Identity

_cached = {}
_last_in_maps = None


def _build():
    nc = bacc.Bacc("TRN2", target_bir_lowering=False, debug=False)

    xta_d = nc.dram_tensor("xta", [C, NH], F32R, kind="ExternalInput").ap()
    xtb_d = nc.dram_tensor("xtb", [C, NH], F32R, kind="ExternalInput").ap()
    wqt_d = nc.dram_tensor("wqt", [C, C], F32R, kind="ExternalInput").ap()
    wkt_d = nc.dram_tensor("wkt", [C, C], F32R, kind="ExternalInput").ap()
    wvt_d = nc.dram_tensor("wvt", [C, C], F32R, kind="ExternalInput").ap()
    bqs_d = nc.dram_tensor("bqs", [P, DC], F32, kind="ExternalInput").ap()
    bks_d = nc.dram_tensor("bks", [P, DC], F32, kind="ExternalInput").ap()
    e_d = nc.dram_tensor("e", [N, NH], F32R, kind="ExternalOutput").ap()
    s_d = nc.dram_tensor("s", [P, QC], F32, kind="ExternalOutput").ap()
    o_d = nc.dram_tensor("o", [N, C], F32, kind="ExternalOutput").ap()

    with tile.TileContext(nc) as tc:
        with (
            tc.tile_pool(name="consts", bufs=1) as consts,
            tc.tile_pool(name="kt", bufs=1) as kt_pool,
            tc.tile_pool(name="v", bufs=1) as v_pool,
            tc.tile_pool(name="qtc", bufs=4) as qtc_pool,
        ):
            xt_stack = ExitStack()
            xta_pool = xt_stack.enter_context(tc.tile_pool(name="xta", bufs=1))
            xtb_pool = xt_stack.enter_context(tc.tile_pool(name="xtb", bufs=1))
            w_pool = xt_stack.enter_context(tc.tile_pool(name="w", bufs=CC))

            ident = consts.tile([P, P], F32R, tag="ident", bufs=1)
            identf = consts.tile([P, P], F32, tag="identf", bufs=1)
            make_identity(nc, identf[:])
            nc.scalar.activation(ident[:], identf[:], IDENT, bias=0.0)
            bq_sb = consts.tile([P, DC], F32, tag="bq", bufs=1)
            nc.sync.dma_start(bq_sb[:], bqs_d[:])
            bk_sb = consts.tile([P, DC], F32, tag="bk", bufs=1)
            nc.sync.dma_start(bk_sb[:], bks_d[:])
            s_all = consts.tile([P, QC], F32, tag="s_all", bufs=1)

            kt_tiles = [kt_pool.tile([P, NH], F32R, name=f"kt{d}") for d in range(DC)]
            v_tiles = [v_pool.tile([P, C], F32R, name=f"v{n}") for n in range(NK8)]

            # loads: own key/query half + wk first (feeds K immediately)
            xta_tiles = []
            for cchunk in range(CC):
                t = xta_pool.tile([P, NH], F32R, name=f"xta{cchunk}")
                nc.sync.dma_start(t[:], xta_d[cchunk * P : (cchunk + 1) * P, :])
                xta_tiles.append(t)
            wk_tiles = []
            for cchunk in range(CC):
                t = w_pool.tile([P, C], F32R, name=f"wk{cchunk}", tag="w")
                nc.sync.dma_start(t[:], wkt_d[cchunk * P : (cchunk + 1) * P, :])
                wk_tiles.append(t)
            late_stack = ExitStack()
            dram = late_stack.enter_context(
                tc.tile_pool(name="dram", bufs=1, space="DRAM")
            )
            qt_scr = [
                dram.tile([P, DC * 512], F32R, name=f"qtscr{qb}") for qb in range(4)
            ]
            qt_scr3 = [t[:].rearrange("p (d n) -> p d n", d=DC) for t in qt_scr]
            bounceq = xt_stack.enter_context(tc.tile_pool(name="bounceq", bufs=4))

            ps1 = late_stack.enter_context(
                tc.tile_pool(name="ps1", bufs=8, space="PSUM")
            )
            if True:
                # ---- K: KT[d, nloc] = WkT.T @ XTA (+bk), resident
                for d in range(DC):
                    pts = [
                        ps1.tile([P, 512], F32, name="p_k", tag="ps1")
                        for _ in range(2)
                    ]
                    for cchunk in range(CC):
                        for nb in range(2):
                            nc.tensor.matmul(
                                pts[nb][:],
                                wk_tiles[cchunk][:, d * P : (d + 1) * P],
                                xta_tiles[cchunk][:, nb * 512 : (nb + 1) * 512],
                                start=(cchunk == 0),
                                stop=(cchunk == CC - 1),
                            )
                    for nb in range(2):
                        nc.scalar.activation(
                            kt_tiles[d][:, nb * 512 : (nb + 1) * 512],
                            pts[nb][:],
                            IDENT,
                            bias=bk_sb[:, d : d + 1],
                        )

                # ---- V: V[nloc, d] = XTA.T @ WvT, resident
                wv_tiles = []
                for cchunk in range(CC):
                    t = w_pool.tile([P, C], F32R, name=f"wv{cchunk}", tag="w")
                    nc.sync.dma_start(t[:], wvt_d[cchunk * P : (cchunk + 1) * P, :])
                    wv_tiles.append(t)
                for n in range(NK8):
                    pts = [
                        ps1.tile([P, 512], F32, name="p_v", tag="ps1")
                        for _ in range(2)
                    ]
                    for cchunk in range(CC):
                        for db in range(2):
                            nc.tensor.matmul(
                                pts[db][:],
                                xta_tiles[cchunk][:, n * P : (n + 1) * P],
                                wv_tiles[cchunk][:, db * 512 : (db + 1) * 512],
                                start=(cchunk == 0),
                                stop=(cchunk == CC - 1),
                            )
                    for db in range(2):
                        nc.vector.tensor_copy(
                            v_tiles[n][:, db * 512 : (db + 1) * 512], pts[db][:]
                        )

                # ---- Q: QT[d, qloc] = WqT.T @ [XTA | XTB] (+bq), resident
                wq_tiles = []
                for cchunk in range(CC):
                    t = w_pool.tile([P, C], F32R, name=f"wq{cchunk}", tag="w")
                    nc.sync.dma_start(t[:], wqt_d[cchunk * P : (cchunk + 1) * P, :])
                    wq_tiles.append(t)
                xtb_tiles = []
                for cchunk in range(CC):
                    t = xtb_pool.tile([P, NH], F32R, name=f"xtb{cchunk}")
                    nc.sync.dma_start(t[:], xtb_d[cchunk * P : (cchunk + 1) * P, :])
                    xtb_tiles.append(t)
                qtc_prefetch = []

                def load_qtc(qc):
                    qb, rel = divmod(qc, 4)
                    t = qtc_pool.tile([P, DC * P], F32R, name="qtc", tag="qtc")
                    nc.sync.dma_start(
                        t[:].rearrange("p (d n) -> p d n", d=DC),
                        qt_scr3[qb][:, :, rel * P : (rel + 1) * P],
                    )
                    return t

                for qb in range(4):
                    x_tiles = xta_tiles if qb < 2 else xtb_tiles
                    qrel = qb % 2
                    for d in range(DC):
                        pt = ps1.tile([P, 512], F32, name="p_q", tag="ps1")
                        for cchunk in range(CC):
                            nc.tensor.matmul(
                                pt[:],
                                wq_tiles[cchunk][:, d * P : (d + 1) * P],
                                x_tiles[cchunk][:, qrel * 512 : (qrel + 1) * 512],
                                start=(cchunk == 0),
                                stop=(cchunk == CC - 1),
                            )
                        qb_t = bounceq.tile([P, 512], F32R, name="qb_t", tag="bq_t")
                        nc.scalar.activation(
                            qb_t[:], pt[:], IDENT, bias=bq_sb[:, d : d + 1]
                        )
                        nc.gpsimd.dma_start(
                            qt_scr[qb][:, d * 512 : (d + 1) * 512], qb_t[:]
                        )
                    if qb == 0:
                        qtc_prefetch = [load_qtc(qc) for qc in range(3)]

            xt_stack.close()

            # ---------- Phase 2: attention, software-pipelined ----------
            with (
                tc.tile_pool(name="a", bufs=4) as a_pool,
                tc.tile_pool(name="atsb", bufs=8) as at_pool,
                tc.tile_pool(name="osb", bufs=3) as o_pool,
                tc.tile_pool(name="small", bufs=16) as small,
            ):
                ps_s = ps_at = ps_o = ps1
                qtc_queue = list(qtc_prefetch)
                prev = None  # (a_sb of chunk i-1, qc index)

                def emit_at(a_sb):
                    at_tiles = []
                    for g in range(2):
                        pt = ps_at.tile([P, 512], F32R, name="p_at", tag="ps1")
                        for j in range(4):
                            kk = g * 4 + j
                            nc.tensor.transpose(
                                pt[:, j * P : (j + 1) * P],
                                a_sb[:, kk * P : (kk + 1) * P],
                                ident[:],
                            )
                        at_sb = at_pool.tile([P, 512], F32R, name="at_sb", tag="at")
                        nc.vector.tensor_copy(at_sb[:], pt[:])
                        at_tiles.append(at_sb)
                    return at_tiles

                def emit_o(at_tiles, qc):
                    o_sb = o_pool.tile([P, C], F32, name="o_sb", tag="o")
                    pts = [
                        ps_o.tile([P, 512], F32, name="p_o", tag="ps1")
                        for _ in range(2)
                    ]
                    for kk in range(NK8):
                        for db in range(2):
                            nc.tensor.matmul(
                                pts[db][:],
                                at_tiles[kk // 4][:, (kk % 4) * P : (kk % 4 + 1) * P],
                                v_tiles[kk][:, db * 512 : (db + 1) * 512],
                                start=(kk == 0),
                                stop=(kk == NK8 - 1),
                            )
                    for db in range(2):
                        nc.scalar.copy(o_sb[:, db * 512 : (db + 1) * 512], pts[db][:])
                    nc.scalar.dma_start(o_d[qc * P : (qc + 1) * P, :], o_sb[:])

                for qc in range(QC):
                    qtc_tiles = qtc_queue.pop(0)
                    # transposes of the previous chunk first: the S matmuls
                    # below cover their PSUM->SBUF copies on the DVE
                    at_prev = emit_at(prev[0]) if prev is not None else None

                    # S_half[q, nloc], exp straight out of PSUM with row sums
                    a_sb = a_pool.tile([P, NH], F32R, name="a_sb", tag="a")
                    s2 = small.tile([P, 2], F32, name="s2", tag="s2")
                    s_pts = [
                        ps_s.tile([P, 512], F32, name="p_s", tag="ps1")
                        for _ in range(2)
                    ]
                    for d in range(DC):
                        for nb in range(2):
                            nc.tensor.matmul(
                                s_pts[nb][:],
                                qtc_tiles[:, d * P : (d + 1) * P],
                                kt_tiles[d][:, nb * 512 : (nb + 1) * 512],
                                start=(d == 0),
                                stop=(d == DC - 1),
                            )
                    if qc + 3 < QC:
                        qtc_queue.append(load_qtc(qc + 3))
                    for nb in range(2):
                        nc.scalar.activation(
                            a_sb[:, nb * 512 : (nb + 1) * 512],
                            s_pts[nb][:],
                            EXP,
                            bias=0.0,
                            accum_out=s2[:, nb : nb + 1],
                        )
                    if at_prev is not None:
                        emit_o(at_prev, prev[1])
                    nc.vector.reduce_sum(
                        s_all[:, qc : qc + 1], s2[:], axis=mybir.AxisListType.X
                    )
                    nc.gpsimd.dma_start(e_d[qc * P : (qc + 1) * P, :], a_sb[:])
                    prev = (a_sb, qc)

                emit_o(emit_at(prev[0]), prev[1])
                nc.sync.dma_start(s_d[:], s_all[:])
            late_stack.close()

    nc.compile()
    return nc


def kernel(hidden_states, Wq, bq, Wk, bk, Wv, bv):
    x = np.asarray(hidden_states, dtype=np.float32)
    Wq = np.asarray(Wq, dtype=np.float32)
    Wk = np.asarray(Wk, dtype=np.float32)
    Wv = np.asarray(Wv, dtype=np.float32)
    bq = np.asarray(bq, dtype=np.float32)
    bk = np.asarray(bk, dtype=np.float32)
    bv = np.asarray(bv, dtype=np.float32)

    if "nc" not in _cached:
        _cached["nc"] = _build()
    nc = _cached["nc"]

    scale = np.float32(1.0 / np.sqrt(C))
    wqt = np.ascontiguousarray(Wq.T) * scale
    wkt = np.ascontiguousarray(Wk.T)
    wvt = np.ascontiguousarray(Wv.T)
    bqs = np.ascontiguousarray((bq * scale).reshape(DC, P).T)
    bks = np.ascontiguousarray(bk.reshape(DC, P).T)

    in_maps = []
    for core in range(8):
        b, kh = divmod(core, 2)
        xt = np.ascontiguousarray(x[b].T)
        mine = xt[:, kh * NH : (kh + 1) * NH]
        other = xt[:, (1 - kh) * NH : (2 - kh) * NH]
        in_maps.append(
            {
                "xta": np.ascontiguousarray(mine),
                "xtb": np.ascontiguousarray(other),
                "wqt": wqt,
                "wkt": wkt,
                "wvt": wvt,
                "bqs": bqs,
                "bks": bks,
            }
        )

    global _last_in_maps
    _last_in_maps = in_maps
    res = run_bass_kernel_spmd(nc, in_maps, core_ids=list(range(8)))

    out = np.empty((B, N, C), dtype=np.float32)
    attention = np.empty((B, N, N), dtype=np.float32)
    for b in range(B):
        r0 = res.results[2 * b]      # kh = 0: local rows = global rows
        r1 = res.results[2 * b + 1]  # kh = 1: local rows = [half1 | half0]
        # s_all[r, qc] holds the row sum for local row qc*128 + r
        s0 = r0["s"].T.reshape(N)
        s1loc = r1["s"].T.reshape(N)
        s1 = np.concatenate([s1loc[NH:], s1loc[:NH]])
        stot = s0 + s1
        e1 = np.concatenate([r1["e"][NH:], r1["e"][:NH]], axis=0)
        attention[b, :, :NH] = r0["e"] / stot[:, None]
        attention[b, :, NH:] = e1 / stot[:, None]
        o1 = np.concatenate([r1["o"][NH:], r1["o"][:NH]], axis=0)
        out[b] = (r0["o"] + o1) / stot[:, None] + bv[None, :]
    return (out, attention)


# revision 20
# speedup vs baseline: 1.0427x; 1.0064x over previous
"""Single-head AttentionBlock (B=4, N=2048, C=1024) on 8 TRN2 NeuronCores.

Key-split sharding: core c handles batch b=c//2 and KEY half kh=c%2.  Each
core computes K^T and V for its 1024 keys only, Q for ALL 2048 query rows,
then unnormalized E = exp(S_half) and O_half = E @ V_half.  The softmax
normalization merges exactly on the host: row sums add across the pair,
attention columns concatenate, partial outputs add then divide.  No
cross-core communication and no duplicated K/V projections.

The host feeds X^T with the core's own key half as columns 0:1024 (xta)
and the partner's as 1024:2048 (xtb); query rows are processed in the same
local order and un-permuted on the host.

All matmuls run in fp32r (full PE rate, ~13-bit operand rounding).
Layouts on chip (partition dim first):
  XTA/XTB = X_b^T halves [c=1024, n=1024]
  KT  = Wk@X^T[:,mine]    [d=1024, nloc=1024]  resident (32KB/part)
  V   = X[mine]@Wv^T      [nloc=1024, d=1024]  resident (32KB/part)
  QT  = Wq@X^T /sqrt d    [d=1024, q=2048]     resident (64KB/part)
  S   = QT.T @ KT         [q, nloc]  -> E = exp(S) from PSUM + row sums
  AT  = PE-transpose(E)   [nloc, q]            (lhsT for out)
  O   = AT.T @ V          [q, d]               unnormalized
bq (pre-scaled by 1/sqrt(d)) and bk are applied via ACT bias on the QT/KT
PSUM->SBUF copies; bv is added on the host.

Everything is SBUF-resident after phase 1 (no DRAM scratch at all).  The
weight pool recycles wk -> wv -> wq(A/B); loads ride Sync+Scalar HWDGE
queues, stores ride Scalar; E/O stores stream per chunk.  Phase 2 is
software-pipelined: iteration i emits S_i/exp_i, then AT_{i-1}/O_{i-1},
so the PE always has work while the (short) exp chain runs on ACT/DVE.
"""
import os

os.environ.pop("JAX_PLATFORMS", None)

from contextlib import ExitStack

import numpy as np

import concourse.bass as bass
import concourse.mybir as mybir
import concourse.tile as tile
from concourse import bacc
from concourse.bass_utils import run_bass_kernel_spmd
from concourse.masks import make_identity

B, N, C = 4, 2048, 1024
NH = N // 2          # key half per core / local key count
P = 128              # partitions
CC = C // P          # 8 contraction chunks
DC = C // P          # 8 d chunks
QC = N // P          # 16 query chunks per core (all rows)
NK8 = NH // P        # 8 local key chunks
F32 = mybir.dt.float32
F32R = mybir.dt.float32r
EXP = mybir.ActivationFunctionType.Exp
IDENT = mybir.ActivationFunctionType.Identity

_cached = {}
_last_in_maps = None


def _build():
    nc = bacc.Bacc("TRN2", target_bir_lowering=False, debug=False)

    xta_d = nc.dram_tensor("xta", [C, NH], F32R, kind="ExternalInput").ap()
    xtb_d = nc.dram_tensor("xtb", [C, NH], F32R, kind="ExternalInput").ap()
    wqt_d = nc.dram_tensor("wqt", [C, C], F32R, kind="ExternalInput").ap()
    wkt_d = nc.dram_tensor("wkt", [C, C], F32R, kind="ExternalInput").ap()
    wvt_d = nc.dram_tensor("wvt", [C, C], F32R, kind="ExternalInput").ap()
    bqs_d = nc.dram_tensor("bqs", [P, DC], F32, kind="ExternalInput").ap()
    bks_d = nc.dram_tensor("bks", [P, DC], F32, kind="ExternalInput").ap()
    e_d = nc.dram_tensor("e", [N, NH], F32R, kind="ExternalOutput").ap()
    s_d = nc.dram_tensor("s", [P, QC], F32, kind="ExternalOutput").ap()
    o_d = nc.dram_tensor("o", [N, C], F32, kind="ExternalOutput").ap()

    with tile.TileContext(nc) as tc:
        with (
            tc.tile_pool(name="consts", bufs=1) as consts,
            tc.tile_pool(name="kt", bufs=1) as kt_pool,
            tc.tile_pool(name="v", bufs=1) as v_pool,
            tc.tile_pool(name="qtc", bufs=4) as qtc_pool,
        ):
            xt_stack = ExitStack()
            xta_pool = xt_stack.enter_context(tc.tile_pool(name="xta", bufs=1))
            xtb_pool = xt_stack.enter_context(tc.tile_pool(name="xtb", bufs=1))
            w_pool = xt_stack.enter_context(tc.tile_pool(name="w", bufs=CC))

            ident = consts.tile([P, P], F32R, tag="ident", bufs=1)
            identf = consts.tile([P, P], F32, tag="identf", bufs=1)
            make_identity(nc, identf[:])
            nc.scalar.activation(ident[:], identf[:], IDENT, bias=0.0)
            bq_sb = consts.tile([P, DC], F32, tag="bq", bufs=1)
            nc.sync.dma_start(bq_sb[:], bqs_d[:])
            bk_sb = consts.tile([P, DC], F32, tag="bk", bufs=1)
            nc.sync.dma_start(bk_sb[:], bks_d[:])
            s_all = consts.tile([P, QC], F32, tag="s_all", bufs=1)

            kt_tiles = [kt_pool.tile([P, NH], F32R, name=f"kt{d}") for d in range(DC)]
            v_tiles = [v_pool.tile([P, C], F32R, name=f"v{n}") for n in range(NK8)]

            # loads: first halves of xta+wk first so K's first PSUM group
            # (nb=0 x d=0, needing xta[:, :512] and wk[:, :512]) starts after
            # only 4MB of input; second halves stream in under K compute
            xta_tiles = [
                xta_pool.tile([P, NH], F32R, name=f"xta{cchunk}")
                for cchunk in range(CC)
            ]
            wk_tiles = [
                w_pool.tile([P, C], F32R, name=f"wk{cchunk}", tag="w")
                for cchunk in range(CC)
            ]
            for cchunk in range(CC):
                nc.sync.dma_start(
                    xta_tiles[cchunk][:, :512],
                    xta_d[cchunk * P : (cchunk + 1) * P, :512],
                )
            for cchunk in range(CC):
                nc.sync.dma_start(
                    wk_tiles[cchunk][:, :512],
                    wkt_d[cchunk * P : (cchunk + 1) * P, :512],
                )
            for cchunk in range(CC):
                nc.sync.dma_start(
                    wk_tiles[cchunk][:, 512:],
                    wkt_d[cchunk * P : (cchunk + 1) * P, 512:],
                )
            for cchunk in range(CC):
                nc.sync.dma_start(
                    xta_tiles[cchunk][:, 512:],
                    xta_d[cchunk * P : (cchunk + 1) * P, 512:],
                )
            late_stack = ExitStack()
            dram = late_stack.enter_context(
                tc.tile_pool(name="dram", bufs=1, space="DRAM")
            )
            qt_scr = [
                dram.tile([P, DC * 512], F32R, name=f"qtscr{qb}") for qb in range(4)
            ]
            qt_scr3 = [t[:].rearrange("p (d n) -> p d n", d=DC) for t in qt_scr]
            bounceq = xt_stack.enter_context(tc.tile_pool(name="bounceq", bufs=4))

            ps1 = late_stack.enter_context(
                tc.tile_pool(name="ps1", bufs=8, space="PSUM")
            )
            if True:
                # ---- K: KT[d, nloc] = WkT.T @ XTA (+bk), resident
                # nb outer: the nb=0 banks only touch the first halves of
                # xta/wk, so K starts after 4MB of input
                for nb in range(2):
                    for d in range(DC):
                        pt = ps1.tile([P, 512], F32, name="p_k", tag="ps1")
                        for cchunk in range(CC):
                            nc.tensor.matmul(
                                pt[:],
                                wk_tiles[cchunk][:, d * P : (d + 1) * P],
                                xta_tiles[cchunk][:, nb * 512 : (nb + 1) * 512],
                                start=(cchunk == 0),
                                stop=(cchunk == CC - 1),
                            )
                        nc.scalar.activation(
                            kt_tiles[d][:, nb * 512 : (nb + 1) * 512],
                            pt[:],
                            IDENT,
                            bias=bk_sb[:, d : d + 1],
                        )

                # ---- V: V[nloc, d] = XTA.T @ WvT, resident
                wv_tiles = []
                for cchunk in range(CC):
                    t = w_pool.tile([P, C], F32R, name=f"wv{cchunk}", tag="w")
                    nc.sync.dma_start(t[:], wvt_d[cchunk * P : (cchunk + 1) * P, :])
                    wv_tiles.append(t)
                for n in range(NK8):
                    pts = [
                        ps1.tile([P, 512], F32, name="p_v", tag="ps1")
                        for _ in range(2)
                    ]
                    for cchunk in range(CC):
                        for db in range(2):
                            nc.tensor.matmul(
                                pts[db][:],
                                xta_tiles[cchunk][:, n * P : (n + 1) * P],
                                wv_tiles[cchunk][:, db * 512 : (db + 1) * 512],
                                start=(cchunk == 0),
                                stop=(cchunk == CC - 1),
                            )
                    for db in range(2):
                        nc.vector.tensor_copy(
                            v_tiles[n][:, db * 512 : (db + 1) * 512], pts[db][:]
                        )

                # ---- Q: QT[d, qloc] = WqT.T @ [XTA | XTB] (+bq), resident
                wq_tiles = []
                for cchunk in range(CC):
                    t = w_pool.tile([P, C], F32R, name=f"wq{cchunk}", tag="w")
                    nc.sync.dma_start(t[:], wqt_d[cchunk * P : (cchunk + 1) * P, :])
                    wq_tiles.append(t)
                xtb_tiles = []
                for cchunk in range(CC):
                    t = xtb_pool.tile([P, NH], F32R, name=f"xtb{cchunk}")
                    nc.sync.dma_start(t[:], xtb_d[cchunk * P : (cchunk + 1) * P, :])
                    xtb_tiles.append(t)
                qtc_prefetch = []

                def load_qtc(qc):
                    qb, rel = divmod(qc, 4)
                    t = qtc_pool.tile([P, DC * P], F32R, name="qtc", tag="qtc")
                    nc.sync.dma_start(
                        t[:].rearrange("p (d n) -> p d n", d=DC),
                        qt_scr3[qb][:, :, rel * P : (rel + 1) * P],
                    )
                    return t

                for qb in range(4):
                    x_tiles = xta_tiles if qb < 2 else xtb_tiles
                    qrel = qb % 2
                    for d in range(DC):
                        pt = ps1.tile([P, 512], F32, name="p_q", tag="ps1")
                        for cchunk in range(CC):
                            nc.tensor.matmul(
                                pt[:],
                                wq_tiles[cchunk][:, d * P : (d + 1) * P],
                                x_tiles[cchunk][:, qrel * 512 : (qrel + 1) * 512],
                                start=(cchunk == 0),
                                stop=(cchunk == CC - 1),
                            )
                        qb_t = bounceq.tile([P, 512], F32R, name="qb_t", tag="bq_t")
                        nc.scalar.activation(
                            qb_t[:], pt[:], IDENT, bias=bq_sb[:, d : d + 1]
                        )
                        nc.gpsimd.dma_start(
                            qt_scr[qb][:, d * 512 : (d + 1) * 512], qb_t[:]
                        )
                    if qb == 0:
                        qtc_prefetch = [load_qtc(qc) for qc in range(3)]

            xt_stack.close()

            # ---------- Phase 2: attention, software-pipelined ----------
            with (
                tc.tile_pool(name="a", bufs=4) as a_pool,
                tc.tile_pool(name="atsb", bufs=8) as at_pool,
                tc.tile_pool(name="osb", bufs=3) as o_pool,
                tc.tile_pool(name="small", bufs=16) as small,
            ):
                ps_s = ps_at = ps_o = ps1
                qtc_queue = list(qtc_prefetch)
                prev = None  # (a_sb of chunk i-1, qc index)

                def emit_at(a_sb):
                    at_tiles = []
                    for g in range(2):
                        pt = ps_at.tile([P, 512], F32R, name="p_at", tag="ps1")
                        for j in range(4):
                            kk = g * 4 + j
                            nc.tensor.transpose(
                                pt[:, j * P : (j + 1) * P],
                                a_sb[:, kk * P : (kk + 1) * P],
                                ident[:],
                            )
                        at_sb = at_pool.tile([P, 512], F32R, name="at_sb", tag="at")
                        nc.vector.tensor_copy(at_sb[:], pt[:])
                        at_tiles.append(at_sb)
                    return at_tiles

                def emit_o(at_tiles, qc):
                    o_sb = o_pool.tile([P, C], F32, name="o_sb", tag="o")
                    pts = [
                        ps_o.tile([P, 512], F32, name="p_o", tag="ps1")
                        for _ in range(2)
                    ]
                    for kk in range(NK8):
                        for db in range(2):
                            nc.tensor.matmul(
                                pts[db][:],
                                at_tiles[kk // 4][:, (kk % 4) * P : (kk % 4 + 1) * P],
                                v_tiles[kk][:, db * 512 : (db + 1) * 512],
                                start=(kk == 0),
                                stop=(kk == NK8 - 1),
                            )
                    for db in range(2):
                        nc.scalar.copy(o_sb[:, db * 512 : (db + 1) * 512], pts[db][:])
                    nc.scalar.dma_start(o_d[qc * P : (qc + 1) * P, :], o_sb[:])

                for qc in range(QC):
                    qtc_tiles = qtc_queue.pop(0)
                    # transposes of the previous chunk first: the S matmuls
                    # below cover their PSUM->SBUF copies on the DVE
                    at_prev = emit_at(prev[0]) if prev is not None else None

                    # S_half[q, nloc], exp straight out of PSUM with row sums
                    a_sb = a_pool.tile([P, NH], F32R, name="a_sb", tag="a")
                    s2 = small.tile([P, 2], F32, name="s2", tag="s2")
                    s_pts = [
                        ps_s.tile([P, 512], F32, name="p_s", tag="ps1")
                        for _ in range(2)
                    ]
                    for d in range(DC):
                        for nb in range(2):
                            nc.tensor.matmul(
                                s_pts[nb][:],
                                qtc_tiles[:, d * P : (d + 1) * P],
                                kt_tiles[d][:, nb * 512 : (nb + 1) * 512],
                                start=(d == 0),
                                stop=(d == DC - 1),
                            )
                    if qc + 3 < QC:
                        qtc_queue.append(load_qtc(qc + 3))
                    for nb in range(2):
                        nc.scalar.activation(
                            a_sb[:, nb * 512 : (nb + 1) * 512],
                            s_pts[nb][:],
                            EXP,
                            bias=0.0,
                            accum_out=s2[:, nb : nb + 1],
                        )
                    if at_prev is not None:
                        emit_o(at_prev, prev[1])
                    nc.vector.reduce_sum(
                        s_all[:, qc : qc + 1], s2[:], axis=mybir.AxisListType.X
                    )
                    nc.gpsimd.dma_start(e_d[qc * P : (qc + 1) * P, :], a_sb[:])
                    prev = (a_sb, qc)

                emit_o(emit_at(prev[0]), prev[1])
                nc.sync.dma_start(s_d[:], s_all[:])
            late_stack.close()

    nc.compile()
    return nc


def kernel(hidden_states, Wq, bq, Wk, bk, Wv, bv):
    x = np.asarray(hidden_states, dtype=np.float32)
    Wq = np.asarray(Wq, dtype=np.float32)
    Wk = np.asarray(Wk, dtype=np.float32)
    Wv = np.asarray(Wv, dtype=np.float32)
    bq = np.asarray(bq, dtype=np.float32)
    bk = np.asarray(bk, dtype=np.float32)
    bv = np.asarray(bv, dtype=np.float32)

    if "nc" not in _cached:
        _cached["nc"] = _build()
    nc = _cached["nc"]

    scale = np.float32(1.0 / np.sqrt(C))
    wqt = np.ascontiguousarray(Wq.T) * scale
    wkt = np.ascontiguousarray(Wk.T)
    wvt = np.ascontiguousarray(Wv.T)
    bqs = np.ascontiguousarray((bq * scale).reshape(DC, P).T)
    bks = np.ascontiguousarray(bk.reshape(DC, P).T)

    in_maps = []
    for core in range(8):
        b, kh = divmod(core, 2)
        xt = np.ascontiguousarray(x[b].T)
        mine = xt[:, kh * NH : (kh + 1) * NH]
        other = xt[:, (1 - kh) * NH : (2 - kh) * NH]
        in_maps.append(
            {
                "xta": np.ascontiguousarray(mine),
                "xtb": np.ascontiguousarray(other),
                "wqt": wqt,
                "wkt": wkt,
                "wvt": wvt,
                "bqs": bqs,
                "bks": bks,
            }
        )

    global _last_in_maps
    _last_in_maps = in_maps
    res = run_bass_kernel_spmd(nc, in_maps, core_ids=list(range(8)))

    out = np.empty((B, N, C), dtype=np.float32)
    attention = np.empty((B, N, N), dtype=np.float32)
    for b in range(B):
        r0 = res.results[2 * b]      # kh = 0: local rows = global rows
        r1 = res.results[2 * b + 1]  # kh = 1: local rows = [half1 | half0]
        # s_all[r, qc] holds the row sum for local row qc*128 + r
        s0 = r0["s"].T.reshape(N)
        s1loc = r1["s"].T.reshape(N)
        s1 = np.concatenate([s1loc[NH:], s1loc[:NH]])
        stot = s0 + s1
        e1 = np.concatenate([r1["e"][NH:], r1["e"][:NH]], axis=0)
        attention[b, :, :NH] = r0["e"] / stot[:, None]
        attention[b, :, NH:] = e1 / stot[:, None]
        o1 = np.concatenate([r1["o"][NH:], r1["o"][:NH]], axis=0)
        out[b] = (r0["o"] + o1) / stot[:, None] + bv[None, :]
    return (out, attention)


# revision 21
# speedup vs baseline: 1.0441x; 1.0014x over previous
"""Single-head AttentionBlock (B=4, N=2048, C=1024) on 8 TRN2 NeuronCores.

Key-split sharding: core c handles batch b=c//2 and KEY half kh=c%2.  Each
core computes K^T and V for its 1024 keys only, Q for ALL 2048 query rows,
then unnormalized E = exp(S_half) and O_half = E @ V_half.  The softmax
normalization merges exactly on the host: row sums add across the pair,
attention columns concatenate, partial outputs add then divide.  No
cross-core communication and no duplicated K/V projections.

The host feeds X^T with the core's own key half as columns 0:1024 (xta)
and the partner's as 1024:2048 (xtb); query rows are processed in the same
local order and un-permuted on the host.

All matmuls run in fp32r (full PE rate, ~13-bit operand rounding).
Layouts on chip (partition dim first):
  XTA/XTB = X_b^T halves [c=1024, n=1024]
  KT  = Wk@X^T[:,mine]    [d=1024, nloc=1024]  resident (32KB/part)
  V   = X[mine]@Wv^T      [nloc=1024, d=1024]  resident (32KB/part)
  QT  = Wq@X^T /sqrt d    [d=1024, q=2048]     resident (64KB/part)
  S   = QT.T @ KT         [q, nloc]  -> E = exp(S) from PSUM + row sums
  AT  = PE-transpose(E)   [nloc, q]            (lhsT for out)
  O   = AT.T @ V          [q, d]               unnormalized
bq (pre-scaled by 1/sqrt(d)) and bk are applied via ACT bias on the QT/KT
PSUM->SBUF copies; bv is added on the host.

K^T and V are SBUF-resident; Q^T bounces through DRAM scratch (one tile
per 512-column block, so per-chunk reloads only depend on the block that
produced them) and streams back per query chunk as a single strided DMA.
The weight pool recycles wk -> wv -> wq; the first halves of xta/wk load
first so the K projection starts after only 4MB of input.  Input loads
ride the Sync HWDGE queue, Q^T/E stores ride the GpSimd SWDGE queue, O
stores ride the Scalar queue.  PSUM comes from one shared 8-bank pool.
Phase 2 is software-pipelined: iteration i emits AT_{i-1} transposes,
then S_i (covering the transpose-drain DVE copies), then exp_i, then
O_{i-1}, so the PE never waits on the softmax chain.
"""
import os

os.environ.pop("JAX_PLATFORMS", None)

from contextlib import ExitStack

import numpy as np

import concourse.bass as bass
import concourse.mybir as mybir
import concourse.tile as tile
from concourse import bacc
from concourse.bass_utils import run_bass_kernel_spmd
from concourse.masks import make_identity

B, N, C = 4, 2048, 1024
NH = N // 2          # key half per core / local key count
P = 128              # partitions
CC = C // P          # 8 contraction chunks
DC = C // P          # 8 d chunks
QC = N // P          # 16 query chunks per core (all rows)
NK8 = NH // P        # 8 local key chunks
F32 = mybir.dt.float32
F32R = mybir.dt.float32r
EXP = mybir.ActivationFunctionType.Exp
IDENT = mybir.ActivationFunctionType.Identity

_cached = {}
_last_in_maps = None


def _build():
    nc = bacc.Bacc("TRN2", target_bir_lowering=False, debug=False)

    xta_d = nc.dram_tensor("xta", [C, NH], F32R, kind="ExternalInput").ap()
    xtb_d = nc.dram_tensor("xtb", [C, NH], F32R, kind="ExternalInput").ap()
    wqt_d = nc.dram_tensor("wqt", [C, C], F32R, kind="ExternalInput").ap()
    wkt_d = nc.dram_tensor("wkt", [C, C], F32R, kind="ExternalInput").ap()
    wvt_d = nc.dram_tensor("wvt", [C, C], F32R, kind="ExternalInput").ap()
    bqs_d = nc.dram_tensor("bqs", [P, DC], F32, kind="ExternalInput").ap()
    bks_d = nc.dram_tensor("bks", [P, DC], F32, kind="ExternalInput").ap()
    e_d = nc.dram_tensor("e", [N, NH], F32R, kind="ExternalOutput").ap()
    s_d = nc.dram_tensor("s", [P, QC], F32, kind="ExternalOutput").ap()
    o_d = nc.dram_tensor("o", [N, C], F32, kind="ExternalOutput").ap()

    with tile.TileContext(nc) as tc:
        with (
            tc.tile_pool(name="consts", bufs=1) as consts,
            tc.tile_pool(name="kt", bufs=1) as kt_pool,
            tc.tile_pool(name="v", bufs=1) as v_pool,
            tc.tile_pool(name="qtc", bufs=4) as qtc_pool,
        ):
            xt_stack = ExitStack()
            xta_pool = xt_stack.enter_context(tc.tile_pool(name="xta", bufs=1))
            xtb_pool = xt_stack.enter_context(tc.tile_pool(name="xtb", bufs=1))
            w_pool = xt_stack.enter_context(tc.tile_pool(name="w", bufs=CC))

            ident = consts.tile([P, P], F32R, tag="ident", bufs=1)
            identf = consts.tile([P, P], F32, tag="identf", bufs=1)
            make_identity(nc, identf[:])
            nc.scalar.activation(ident[:], identf[:], IDENT, bias=0.0)
            bq_sb = consts.tile([P, DC], F32, tag="bq", bufs=1)
            nc.sync.dma_start(bq_sb[:], bqs_d[:])
            bk_sb = consts.tile([P, DC], F32, tag="bk", bufs=1)
            nc.sync.dma_start(bk_sb[:], bks_d[:])
            s_all = consts.tile([P, QC], F32, tag="s_all", bufs=1)

            kt_tiles = [kt_pool.tile([P, NH], F32R, name=f"kt{d}") for d in range(DC)]
            v_tiles = [v_pool.tile([P, C], F32R, name=f"v{n}") for n in range(NK8)]

            # loads: first halves of xta+wk first so K's first PSUM group
            # (nb=0 x d=0, needing xta[:, :512] and wk[:, :512]) starts after
            # only 4MB of input; second halves stream in under K compute
            xta_tiles = [
                xta_pool.tile([P, NH], F32R, name=f"xta{cchunk}")
                for cchunk in range(CC)
            ]
            wk_tiles = [
                w_pool.tile([P, C], F32R, name=f"wk{cchunk}", tag="w")
                for cchunk in range(CC)
            ]
            for cchunk in range(CC):
                nc.sync.dma_start(
                    xta_tiles[cchunk][:, :512],
                    xta_d[cchunk * P : (cchunk + 1) * P, :512],
                )
            for cchunk in range(CC):
                nc.sync.dma_start(
                    wk_tiles[cchunk][:, :512],
                    wkt_d[cchunk * P : (cchunk + 1) * P, :512],
                )
            for cchunk in range(CC):
                nc.sync.dma_start(
                    wk_tiles[cchunk][:, 512:],
                    wkt_d[cchunk * P : (cchunk + 1) * P, 512:],
                )
            for cchunk in range(CC):
                nc.sync.dma_start(
                    xta_tiles[cchunk][:, 512:],
                    xta_d[cchunk * P : (cchunk + 1) * P, 512:],
                )
            late_stack = ExitStack()
            dram = late_stack.enter_context(
                tc.tile_pool(name="dram", bufs=1, space="DRAM")
            )
            qt_scr = [
                dram.tile([P, DC * 512], F32R, name=f"qtscr{qb}") for qb in range(4)
            ]
            qt_scr3 = [t[:].rearrange("p (d n) -> p d n", d=DC) for t in qt_scr]
            bounceq = xt_stack.enter_context(tc.tile_pool(name="bounceq", bufs=4))

            ps1 = late_stack.enter_context(
                tc.tile_pool(name="ps1", bufs=8, space="PSUM")
            )
            if True:
                # ---- K: KT[d, nloc] = WkT.T @ XTA (+bk), resident
                # nb outer: the nb=0 banks only touch the first halves of
                # xta/wk, so K starts after 4MB of input
                for nb in range(2):
                    for d in range(DC):
                        pt = ps1.tile([P, 512], F32, name="p_k", tag="ps1")
                        for cchunk in range(CC):
                            nc.tensor.matmul(
                                pt[:],
                                wk_tiles[cchunk][:, d * P : (d + 1) * P],
                                xta_tiles[cchunk][:, nb * 512 : (nb + 1) * 512],
                                start=(cchunk == 0),
                                stop=(cchunk == CC - 1),
                            )
                        nc.scalar.activation(
                            kt_tiles[d][:, nb * 512 : (nb + 1) * 512],
                            pt[:],
                            IDENT,
                            bias=bk_sb[:, d : d + 1],
                        )

                # ---- V: V[nloc, d] = XTA.T @ WvT, resident
                wv_tiles = []
                for cchunk in range(CC):
                    t = w_pool.tile([P, C], F32R, name=f"wv{cchunk}", tag="w")
                    nc.sync.dma_start(t[:], wvt_d[cchunk * P : (cchunk + 1) * P, :])
                    wv_tiles.append(t)
                for n in range(NK8):
                    pts = [
                        ps1.tile([P, 512], F32, name="p_v", tag="ps1")
                        for _ in range(2)
                    ]
                    for cchunk in range(CC):
                        for db in range(2):
                            nc.tensor.matmul(
                                pts[db][:],
                                xta_tiles[cchunk][:, n * P : (n + 1) * P],
                                wv_tiles[cchunk][:, db * 512 : (db + 1) * 512],
                                start=(cchunk == 0),
                                stop=(cchunk == CC - 1),
                            )
                    for db in range(2):
                        nc.vector.tensor_copy(
                            v_tiles[n][:, db * 512 : (db + 1) * 512], pts[db][:]
                        )

                # ---- Q: QT[d, qloc] = WqT.T @ [XTA | XTB] (+bq), resident
                wq_tiles = []
                for cchunk in range(CC):
                    t = w_pool.tile([P, C], F32R, name=f"wq{cchunk}", tag="w")
                    nc.sync.dma_start(t[:], wqt_d[cchunk * P : (cchunk + 1) * P, :])
                    wq_tiles.append(t)
                xtb_tiles = []
                for cchunk in range(CC):
                    t = xtb_pool.tile([P, NH], F32R, name=f"xtb{cchunk}")
                    nc.sync.dma_start(t[:], xtb_d[cchunk * P : (cchunk + 1) * P, :])
                    xtb_tiles.append(t)
                qtc_prefetch = []

                def load_qtc(qc):
                    qb, rel = divmod(qc, 4)
                    t = qtc_pool.tile([P, DC * P], F32R, name="qtc", tag="qtc")
                    nc.sync.dma_start(
                        t[:].rearrange("p (d n) -> p d n", d=DC),
                        qt_scr3[qb][:, :, rel * P : (rel + 1) * P],
                    )
                    return t

                for qb in range(4):
                    x_tiles = xta_tiles if qb < 2 else xtb_tiles
                    qrel = qb % 2
                    for d in range(DC):
                        pt = ps1.tile([P, 512], F32, name="p_q", tag="ps1")
                        for cchunk in range(CC):
                            nc.tensor.matmul(
                                pt[:],
                                wq_tiles[cchunk][:, d * P : (d + 1) * P],
                                x_tiles[cchunk][:, qrel * 512 : (qrel + 1) * 512],
                                start=(cchunk == 0),
                                stop=(cchunk == CC - 1),
                            )
                        qb_t = bounceq.tile([P, 512], F32R, name="qb_t", tag="bq_t")
                        nc.scalar.activation(
                            qb_t[:], pt[:], IDENT, bias=bq_sb[:, d : d + 1]
                        )
                        nc.gpsimd.dma_start(
                            qt_scr[qb][:, d * 512 : (d + 1) * 512], qb_t[:]
                        )
                    if qb == 0:
                        qtc_prefetch = [load_qtc(qc) for qc in range(3)]

            xt_stack.close()

            # ---------- Phase 2: attention, software-pipelined ----------
            with (
                tc.tile_pool(name="a", bufs=4) as a_pool,
                tc.tile_pool(name="atsb", bufs=8) as at_pool,
                tc.tile_pool(name="osb", bufs=3) as o_pool,
                tc.tile_pool(name="small", bufs=16) as small,
            ):
                ps_s = ps_at = ps_o = ps1
                qtc_queue = list(qtc_prefetch)
                prev = None  # (a_sb of chunk i-1, qc index)

                def emit_at(a_sb):
                    at_tiles = []
                    for g in range(2):
                        pt = ps_at.tile([P, 512], F32R, name="p_at", tag="ps1")
                        for j in range(4):
                            kk = g * 4 + j
                            nc.tensor.transpose(
                                pt[:, j * P : (j + 1) * P],
                                a_sb[:, kk * P : (kk + 1) * P],
                                ident[:],
                            )
                        at_sb = at_pool.tile([P, 512], F32R, name="at_sb", tag="at")
                        nc.vector.tensor_copy(at_sb[:], pt[:])
                        at_tiles.append(at_sb)
                    return at_tiles

                def emit_o(at_tiles, qc):
                    o_sb = o_pool.tile([P, C], F32, name="o_sb", tag="o")
                    pts = [
                        ps_o.tile([P, 512], F32, name="p_o", tag="ps1")
                        for _ in range(2)
                    ]
                    for kk in range(NK8):
                        for db in range(2):
                            nc.tensor.matmul(
                                pts[db][:],
                                at_tiles[kk // 4][:, (kk % 4) * P : (kk % 4 + 1) * P],
                                v_tiles[kk][:, db * 512 : (db + 1) * 512],
                                start=(kk == 0),
                                stop=(kk == NK8 - 1),
                            )
                    for db in range(2):
                        nc.scalar.copy(o_sb[:, db * 512 : (db + 1) * 512], pts[db][:])
                    nc.scalar.dma_start(o_d[qc * P : (qc + 1) * P, :], o_sb[:])

                for qc in range(QC):
                    qtc_tiles = qtc_queue.pop(0)
                    # transposes of the previous chunk first: the S matmuls
                    # below cover their PSUM->SBUF copies on the DVE
                    at_prev = emit_at(prev[0]) if prev is not None else None

                    # S_half[q, nloc], exp straight out of PSUM with row sums
                    a_sb = a_pool.tile([P, NH], F32R, name="a_sb", tag="a")
                    s2 = small.tile([P, 2], F32, name="s2", tag="s2")
                    s_pts = [
                        ps_s.tile([P, 512], F32, name="p_s", tag="ps1")
                        for _ in range(2)
                    ]
                    for d in range(DC):
                        for nb in range(2):
                            nc.tensor.matmul(
                                s_pts[nb][:],
                                qtc_tiles[:, d * P : (d + 1) * P],
                                kt_tiles[d][:, nb * 512 : (nb + 1) * 512],
                                start=(d == 0),
                                stop=(d == DC - 1),
                            )
                    if qc + 3 < QC:
                        qtc_queue.append(load_qtc(qc + 3))
                    for nb in range(2):
                        nc.scalar.activation(
                            a_sb[:, nb * 512 : (nb + 1) * 512],
                            s_pts[nb][:],
                            EXP,
                            bias=0.0,
                            accum_out=s2[:, nb : nb + 1],
                        )
                    if at_prev is not None:
                        emit_o(at_prev, prev[1])
                    nc.vector.reduce_sum(
                        s_all[:, qc : qc + 1], s2[:], axis=mybir.AxisListType.X
                    )
                    nc.gpsimd.dma_start(e_d[qc * P : (qc + 1) * P, :], a_sb[:])
                    prev = (a_sb, qc)

                emit_o(emit_at(prev[0]), prev[1])
                nc.sync.dma_start(s_d[:], s_all[:])
            late_stack.close()

    nc.compile()
    return nc


def kernel(hidden_states, Wq, bq, Wk, bk, Wv, bv):
    x = np.asarray(hidden_states, dtype=np.float32)
    Wq = np.asarray(Wq, dtype=np.float32)
    Wk = np.asarray(Wk, dtype=np.float32)
    Wv = np.asarray(Wv, dtype=np.float32)
    bq = np.asarray(bq, dtype=np.float32)
    bk = np.asarray(bk, dtype=np.float32)
    bv = np.asarray(bv, dtype=np.float32)

    if "nc" not in _cached:
        _cached["nc"] = _build()
    nc = _cached["nc"]

    scale = np.float32(1.0 / np.sqrt(C))
    wqt = np.ascontiguousarray(Wq.T) * scale
    wkt = np.ascontiguousarray(Wk.T)
    wvt = np.ascontiguousarray(Wv.T)
    bqs = np.ascontiguousarray((bq * scale).reshape(DC, P).T)
    bks = np.ascontiguousarray(bk.reshape(DC, P).T)

    in_maps = []
    for core in range(8):
        b, kh = divmod(core, 2)
        xt = np.ascontiguousarray(x[b].T)
        mine = xt[:, kh * NH : (kh + 1) * NH]
        other = xt[:, (1 - kh) * NH : (2 - kh) * NH]
        in_maps.append(
            {
                "xta": np.ascontiguousarray(mine),
                "xtb": np.ascontiguousarray(other),
                "wqt": wqt,
                "wkt": wkt,
                "wvt": wvt,
                "bqs": bqs,
                "bks": bks,
            }
        )

    global _last_in_maps
    _last_in_maps = in_maps
    res = run_bass_kernel_spmd(nc, in_maps, core_ids=list(range(8)))

    out = np.empty((B, N, C), dtype=np.float32)
    attention = np.empty((B, N, N), dtype=np.float32)
    for b in range(B):
        r0 = res.results[2 * b]      # kh = 0: local rows = global rows
        r1 = res.results[2 * b + 1]  # kh = 1: local rows = [half1 | half0]
        # s_all[r, qc] holds the row sum for local row qc*128 + r
        s0 = r0["s"].T.reshape(N)
        s1loc = r1["s"].T.reshape(N)
        s1 = np.concatenate([s1loc[NH:], s1loc[:NH]])
        stot = s0 + s1
        e1 = np.concatenate([r1["e"][NH:], r1["e"][:NH]], axis=0)
        attention[b, :, :NH] = r0["e"] / stot[:, None]
        attention[b, :, NH:] = e1 / stot[:, None]
        o1 = np.concatenate([r1["o"][NH:], r1["o"][:NH]], axis=0)
        out[b] = (r0["o"] + o1) / stot[:, None] + bv[None, :]
    return (out, attention)
